# revision 10
# baseline (speedup 1.0000x reference)
"""Trainium2 Bass kernel for nn_CollapsedPBFAOptimized (Chebyshev kernelized
linear attention), bf16 fast path.

Sharding (8 cores): core c handles batch b = c//4 and the 4 heads
[4*(c%4) .. 4*(c%4)+3].  Each core computes a partial output
(x[b] @ w_in_sub -> features -> per-head KV state -> out rows) projected
through its w_out columns; the host sums the 4 partials per batch.

Math: the collapsed beta is zero for m=0 and m>=6, so the kernel is
  out_i = sum_{m=1..5} beta_m <T_m(q_i), T_m(k_j)> v_j
with only FIVE Chebyshev features per dim.  Features are computed with
scaled one-op recurrences (f1=t, f2=t^2-1/2, f3=(t^2-3/4)t, f4=f2^2-1/8,
f5=f2*f3-t/16 => f_m = T_m/2^(m-1)); the scale correction
beta_m * 4^(m-1) is folded into the block-diagonal weight tiles for the
query-side contraction.

Schedule: the per-head KV-state matmuls (phase 2) are interleaved into
phase 1's PE stream with a one-ss delay (features for a chunk are
computed on DVE/ACT while the PE works on the next ss), using an
8-chunk rotating window for k-features and v.  Query features are also
precomputed during phase 1, so the final phase is pure matmul + PSUM
eviction.  All matmuls run in bf16 (1 PE cycle/row vs 4 for fp32);
PSUM accumulates fp32.
"""
import json
import sys
import numpy as np
from contextlib import ExitStack
from functools import lru_cache

sys.path.insert(0, '/opt/trn_rl_repo')

import concourse.bass as bass
import concourse.tile as tile
from concourse import mybir, bass_utils

# ---------------------------------------------------------------------------
# Toolchain patches (walrus on this image supports one sync-wait per inst)
# ---------------------------------------------------------------------------


def _install_patches():
    from concourse.tile import ScopedClock
    from concourse import bass2jax

    def _patched_drain_and_barrier(self, tick_clock, wait_clock):
        drain_inst = self.nc.sync.drain()
        wait_clock.add_sem_waits(
            drain_inst.ins, ScopedClock({None: tick_clock.global_clock}))
        si = drain_inst.ins.sync_info
        if si is not None:
            w = list(si.on_wait)
            if len(w) > 1:
                si.on_wait = [w[0]]
                for extra in w[1:]:
                    d2 = self.nc.sync.drain()
                    d2.ins.sync_info = mybir.SyncInfo(on_wait=[extra], on_update=[])
        self.nc.all_engine_barrier()
        assert self.sems is not None
        popped = self.nc._tile_sem_poison_stack.pop()
        assert popped is self._sem_poison
        self.nc.clear_and_free_semaphores(list(self.sems.allocated().values()))
        self.nc.all_engine_barrier()

    tile.TileContext._drain_and_barrier = _patched_drain_and_barrier

    LIMIT = 1

    def split_waits_in_bir_json(bir_json):
        d = json.loads(bir_json.decode() if isinstance(bir_json, bytes) else bir_json)
        for fn in d.get('functions', []):
            for bb in fn.get('blocks', []):
                out, changed = [], False
                for ins in bb.get('instructions', []):
                    si = ins.get('sync_info')
                    waits = (si or {}).get('on_wait') or []
                    if len(waits) > LIMIT:
                        for k, w in enumerate(waits[:-LIMIT]):
                            nop = {'name': ins['name'] + f'-xw{k}',
                                   'engine': ins['engine'], 'opcode': 'NoOp',
                                   'ins': [], 'outs': [],
                                   'sync_info': {'on_wait': [w], 'on_update': []}}
                            if 'debug' in ins:
                                nop['debug'] = ins['debug']
                            out.append(nop)
                        si['on_wait'] = waits[-LIMIT:]
                        changed = True
                    out.append(ins)
                if changed:
                    bb['instructions'] = out
        return json.dumps(d).encode()

    if not getattr(bass_utils.compile_bir_kernel, '_wait_patched', False):
        orig = bass_utils.compile_bir_kernel

        def patched(bir_json, tmpdir, neff_name='file.neff'):
            return orig(split_waits_in_bir_json(bir_json), tmpdir, neff_name)

        patched._wait_patched = True
        bass_utils.compile_bir_kernel = patched
        bass2jax.compile_bir_kernel = patched


_install_patches()

# ---------------------------------------------------------------------------
# Problem constants (hardcoded per the task contract)
# ---------------------------------------------------------------------------
B, S, D = 2, 4096, 1024
H, DH = 16, 64
NF = 5                       # Chebyshev features T_1..T_5 (beta_0 = beta_{6..} = 0)
SCALE = DH ** -0.5
HPC = 4                      # heads per core
NCORES = 8
F32 = mybir.dt.float32
BF16 = mybir.dt.bfloat16
NCHUNK = S // 128            # 32 seq chunks of 128
FTC = HPC * NF * 64          # FT columns per chunk slot = 1280
NSLOT = 8                    # rotating chunk window (k features / v)

# ---------------------------------------------------------------------------
# Device program
# ---------------------------------------------------------------------------


def _build_program():
    nc = bass.Bass('TRN2', target_bir_lowering=False, debug=False,
                   num_devices=NCORES)
    ap = {}
    ap['xT'] = nc.dram_tensor('xT', (D, S), BF16, kind='ExternalInput').ap()
    ap['wqT'] = nc.dram_tensor('wqT', (D, 256), BF16, kind='ExternalInput').ap()
    ap['wkvT'] = nc.dram_tensor('wkvT', (D, 512), BF16, kind='ExternalInput').ap()
    ap['woT'] = nc.dram_tensor('woT', (256, D), BF16, kind='ExternalInput').ap()
    ap['betac'] = nc.dram_tensor('betac', (128, 20), F32, kind='ExternalInput').ap()
    ap['eye'] = nc.dram_tensor('eye', (64, 64), BF16, kind='ExternalInput').ap()
    ap['outp'] = nc.dram_tensor('outp', (D, S), BF16, kind='ExternalOutput').ap()
    import os
    ap['_debug'] = os.environ.get('KBDBG', '') == '1'
    if ap['_debug']:
        ap['dbg_qb'] = nc.dram_tensor('dbg_qb', (2, 128, S), BF16, kind='ExternalOutput').ap()
        ap['dbg_qf'] = nc.dram_tensor('dbg_qf', (8, 128, S), BF16, kind='ExternalOutput').ap()
        ap['dbg_wm'] = nc.dram_tensor('dbg_wm', (10, 128, 128), BF16, kind='ExternalOutput').ap()
        ap['dbg_oT'] = nc.dram_tensor('dbg_oT', (2, 128, S), BF16, kind='ExternalOutput').ap()

    with tile.TileContext(nc) as tc:
        with ExitStack() as ctx:
            _emit(nc, tc, ctx, ap)
    return nc


def _emit(nc, tc, ctx, ap):
    TS = mybir.AluOpType
    SQ = mybir.ActivationFunctionType.Square

    const = ctx.enter_context(tc.tile_pool(name='const', bufs=1))
    persist = ctx.enter_context(tc.tile_pool(name='persist', bufs=1))

    betac_sb = const.tile([128, 20], F32, tag='betac', name='betac')
    nc.sync.dma_start(betac_sb[:], ap['betac'][:])
    eye_sb = const.tile([64, 64], BF16, tag='eye', name='eye')
    nc.sync.dma_start(eye_sb[:], ap['eye'][:])

    qb = [persist.tile([128, S], BF16, tag=f'qb{hp}', name=f'qb{hp}') for hp in range(2)]
    # q features f2..f5 per head-pair, full length (consumed in phase 4)
    qf = [[persist.tile([128, S], BF16, tag=f'qf{hp}_{f}', name=f'qf{hp}_{f}')
           for f in range(4)] for hp in range(2)]
    FT = persist.tile([128, NSLOT * FTC], BF16, tag='FT', name='FT')
    v_all = persist.tile([128, NSLOT * 256], BF16, tag='v_all', name='v_all')
    # FT column layout: slot*1280 + hl*320 + m*64 + d   (m = 0..4 <-> T_1..T_5)
    ftv = FT[:].rearrange("p (c h m d) -> p c h m d", h=HPC, m=NF, d=64)

    # block-diagonal weight tiles for phase 4 (zeroed early; filled in ph3)
    Wm = [[persist.tile([128, 128], BF16, tag=f'wm{hp}_{m}', name=f'wm{hp}_{m}')
           for m in range(NF)] for hp in range(2)]
    for hp in range(2):
        for m in range(NF):
            nc.gpsimd.memset(Wm[hp][m][:], 0.0)

    # ---------------- Phase 1 + 2 interleaved -----------------------------
    def emit_features(pool, f1, f2, f3, f4, f5, width):
        """f1 (=clamped input) -> f2..f5 feature views; scratch from pool."""
        hsq = pool.tile([128, width], BF16, tag='hsq', name='hsq')
        s4t = pool.tile([128, width], BF16, tag='s4t', name='s4t')
        t5t = pool.tile([128, width], BF16, tag='t5t', name='t5t')
        nc.scalar.activation(hsq[:], f1, SQ)
        nc.vector.tensor_scalar(f2, hsq[:], -0.5, None, op0=TS.add)
        nc.vector.scalar_tensor_tensor(f3, hsq[:], -0.75, f1,
                                       op0=TS.add, op1=TS.mult)
        nc.scalar.activation(s4t[:], f2, SQ)
        nc.vector.tensor_scalar(f4, s4t[:], -0.125, None, op0=TS.add)
        nc.vector.tensor_tensor(t5t[:], f2, f3, op=TS.mult)
        nc.vector.scalar_tensor_tensor(f5, f1, -0.0625, t5t[:],
                                       op0=TS.mult, op1=TS.add)

    pst = None
    with tc.tile_pool(name='ph1w', bufs=1) as ph1w, \
         tc.tile_pool(name='xt', bufs=2) as xtp, \
         tc.tile_pool(name='fsc', bufs=2) as fscp, \
         tc.tile_pool(name='ps1', bufs=2, space='PSUM') as ps1, \
         tc.tile_pool(name='ps2', bufs=1, space='PSUM') as ps2:
        # k/v state accumulators: one PSUM bank per head, open all of ph1
        pst = [ps2.tile([64, 320], F32, tag=f'pst{h}', name=f'pst{h}')
               for h in range(HPC)]

        # DMA order: x(ss=0) first (split per tile across two queues for
        # startup latency), then wq, then wkv (kv matmuls run after q)
        xt_next = []
        for i in range(8):
            t = xtp.tile([128, 512], BF16, tag=f'xt{i}', name=f'xt{i}')
            nc.sync.dma_start(t[0:64, :], ap['xT'][i * 128:i * 128 + 64, 0:512])
            nc.sync.dma_start(t[64:128, :], ap['xT'][i * 128 + 64:(i + 1) * 128, 0:512])
            xt_next.append(t)
        wq_sb = []
        for i in range(8):
            wq = ph1w.tile([128, 256], BF16, tag=f'wq{i}', name=f'wq{i}')
            nc.sync.dma_start(wq[:], ap['wqT'][i * 128:(i + 1) * 128, :])
            wq_sb.append(wq)
        wkv_sb = []
        for i in range(8):
            wkv = ph1w.tile([128, 512], BF16, tag=f'wkv{i}', name=f'wkv{i}')
            nc.sync.dma_start(wkv[:], ap['wkvT'][i * 128:(i + 1) * 128, :])
            wkv_sb.append(wkv)

        def emit_p2(ss, scs=(0, 1, 2, 3)):
            # state matmuls for chunks produced by ss (features ready)
            for sc in scs:
                c = ss * 4 + sc
                slot = c % NSLOT
                for h in range(HPC):
                    nc.tensor.matmul(
                        pst[h][:],
                        v_all[:, slot * 256 + h * 64:slot * 256 + (h + 1) * 64],
                        FT[:, slot * FTC + h * 320:slot * FTC + (h + 1) * 320],
                        start=(c == 0), stop=(c == NCHUNK - 1))

        for ss in range(8):
            xt = xt_next
            if ss < 7:
                xt_next = []
                for i in range(8):
                    t = xtp.tile([128, 512], BF16, tag=f'xt{i}', name=f'xt{i}')
                    nc.sync.dma_start(t[:], ap['xT'][i * 128:(i + 1) * 128,
                                                     (ss + 1) * 512:(ss + 2) * 512])
                    xt_next.append(t)
            for hp in range(2):
                pq = ps1.tile([128, 512], F32, tag='pq', name='pq')
                for i in range(8):
                    nc.tensor.matmul(pq[:], wq_sb[i][:, hp * 128:(hp + 1) * 128],
                                     xt[i][:], start=(i == 0), stop=(i == 7))
                nc.vector.tensor_scalar(qb[hp][:, ss * 512:(ss + 1) * 512], pq[:],
                                        -1.0, 1.0, op0=TS.max, op1=TS.min)
            for sc in range(4):
                pkv = ps1.tile([128, 512], F32, tag='pkv', name='pkv')
                for i in range(8):
                    nc.tensor.matmul(pkv[:], xt[i][:, sc * 128:(sc + 1) * 128],
                                     wkv_sb[i][:], start=(i == 0), stop=(i == 7))
                c = ss * 4 + sc
                slot = c % NSLOT
                nc.vector.tensor_scalar(ftv[:, slot, :, 0, :], pkv[:, 0:256],
                                        -1.0, 1.0, op0=TS.max, op1=TS.min)
                nc.scalar.copy(v_all[:, slot * 256:(slot + 1) * 256],
                               pkv[:, 256:512])
            # delayed phase-2 matmuls for the previous ss
            if ss > 0:
                emit_p2(ss - 1)
            # k features for this ss's chunks (2 pairs)
            for gp in range(2):
                s0 = (ss * 4 + 2 * gp) % NSLOT
                emit_features(fscp,
                              ftv[:, s0:s0 + 2, :, 0, :],
                              ftv[:, s0:s0 + 2, :, 1, :],
                              ftv[:, s0:s0 + 2, :, 2, :],
                              ftv[:, s0:s0 + 2, :, 3, :],
                              ftv[:, s0:s0 + 2, :, 4, :], 512)
                if ss == 7:
                    emit_p2(7, (2 * gp, 2 * gp + 1))
            # q features for this ss
            for hp in range(2):
                sl = slice(ss * 512, (ss + 1) * 512)
                emit_features(fscp, qb[hp][:, sl],
                              qf[hp][0][:, sl], qf[hp][1][:, sl],
                              qf[hp][2][:, sl], qf[hp][3][:, sl], 512)
        # evict the state accumulators while ps2 is still alive
        pstsb = []
        for h in range(HPC):
            t = persist.tile([64, 320], BF16, tag=f'pstsb{h}', name=f'pstsb{h}')
            if h % 2 == 0:
                nc.scalar.copy(t[:], pst[h][:])
            else:
                nc.vector.tensor_copy(t[:], pst[h][:])
            pstsb.append(t)

    # ---------------- Phase 3: transpose state into Wm --------------------
    with tc.tile_pool(name='psT', bufs=2, space='PSUM') as psTp:
        for hp in range(2):
            for who in range(2):
                h = 2 * hp + who
                for j in range(3):
                    w = 128 if j < 2 else 64
                    pT = psTp.tile([128, 64], BF16, tag='pT', name='pT')
                    nc.tensor.transpose(pT[0:w, :],
                                        pstsb[h][:, j * 128:j * 128 + w],
                                        eye_sb[:])
                    for half in range(2):
                        m = 2 * j + half
                        if m >= NF:
                            continue
                        dst = Wm[hp][m][who * 64:(who + 1) * 64,
                                        who * 64:(who + 1) * 64]
                        src = pT[half * 64:half * 64 + 64, :]
                        sca = betac_sb[half * 64:half * 64 + 64,
                                       hp * 10 + who * 5 + m:hp * 10 + who * 5 + m + 1]
                        if who == 0:
                            nc.vector.tensor_scalar(dst, src, sca, None, op0=TS.mult)
                        else:
                            nc.scalar.activation(
                                dst, src, mybir.ActivationFunctionType.Copy,
                                bias=0.0, scale=sca)

    if ap['_debug']:
        for hp in range(2):
            nc.sync.dma_start(ap['dbg_qb'][hp], qb[hp][:])
            for f in range(4):
                nc.sync.dma_start(ap['dbg_qf'][hp * 4 + f], qf[hp][f][:])
            for m in range(NF):
                nc.sync.dma_start(ap['dbg_wm'][hp * NF + m], Wm[hp][m][:])

    # ---------------- Phase 4: q contraction + out projection -------------
    outT = [persist.tile([128, S], BF16, tag=f'outT{hp}', name=f'outT{hp}')
            for hp in range(2)]
    with tc.tile_pool(name='ph4w', bufs=1) as ph4w, \
         tc.tile_pool(name='osb', bufs=3) as osbp, \
         tc.tile_pool(name='ps4', bufs=4, space='PSUM') as ps4, \
         tc.tile_pool(name='ps5', bufs=3, space='PSUM') as ps5:
        wo_sb = []
        for kc in range(2):
            w = ph4w.tile([128, D], BF16, tag=f'wo{kc}', name=f'wo{kc}')
            nc.sync.dma_start(w[:], ap['woT'][kc * 128:(kc + 1) * 128, :])
            wo_sb.append(w)

        def emit_p4a(t):
            sl = slice(t * 512, (t + 1) * 512)
            for hp in range(2):
                rhs = [qb[hp][:, sl]] + [qf[hp][f][:, sl] for f in range(4)]
                pO = ps4.tile([128, 512], F32, tag='pO', name='pO')
                for m in range(NF):
                    nc.tensor.matmul(pO[:], Wm[hp][m][:], rhs[m],
                                     start=(m == 0), stop=(m == NF - 1))
                dst = outT[hp][:, sl]
                if hp == 0:
                    nc.scalar.copy(dst, pO[:])
                else:
                    nc.vector.tensor_copy(dst, pO[:])

        def emit_p4b(t):
            sl = slice(t * 512, (t + 1) * 512)
            for o in range(8):
                pP = ps5.tile([128, 512], F32, tag='pP', name='pP')
                nc.tensor.matmul(pP[:], wo_sb[0][:, o * 128:(o + 1) * 128],
                                 outT[0][:, sl], start=True, stop=False)
                nc.tensor.matmul(pP[:], wo_sb[1][:, o * 128:(o + 1) * 128],
                                 outT[1][:, sl], start=False, stop=True)
                ob = osbp.tile([128, 512], BF16, tag='ob', name='ob')
                if o % 2 == 0:
                    nc.scalar.copy(ob[:], pP[:])
                else:
                    nc.vector.tensor_copy(ob[:], pP[:])
                nc.sync.dma_start(ap['outp'][o * 128:(o + 1) * 128, sl], ob[:])

        # software pipeline: P4a runs one tile ahead of P4b so the outT
        # evictions complete while the PE works on the next tile's P4a
        emit_p4a(0)
        for t in range(8):
            if t + 1 < 8:
                emit_p4a(t + 1)
            emit_p4b(t)

    if ap['_debug']:
        for hp in range(2):
            nc.sync.dma_start(ap['dbg_oT'][hp], outT[hp][:])


@lru_cache(maxsize=1)
def _get_program():
    return _build_program()


# ---------------------------------------------------------------------------
# Host-side packing
# ---------------------------------------------------------------------------

last_results = None


def kernel(x, w_in, w_out, beta):
    import ml_dtypes
    bf16 = ml_dtypes.bfloat16

    x = np.asarray(x, dtype=np.float32)
    w_in = np.asarray(w_in, dtype=np.float32)
    w_out = np.asarray(w_out, dtype=np.float32)
    beta = np.asarray(beta, dtype=np.float32)
    nc = _get_program()

    xT = [np.ascontiguousarray(x[b].T).astype(bf16) for b in range(B)]
    eye = np.eye(64, dtype=bf16)
    # beta' = beta_{m+1} * 4^m  (feature scales f_m = T_m / 2^(m-1))
    pow4 = (4.0 ** np.arange(NF)).astype(np.float32)
    in_maps = []
    for cid in range(NCORES):
        b, hg = cid // 4, cid % 4
        heads = [4 * hg + j for j in range(HPC)]
        wqT = np.empty((D, 256), dtype=np.float32)
        wkvT = np.empty((D, 512), dtype=np.float32)
        for hl, h in enumerate(heads):
            wqT[:, hl * 64:(hl + 1) * 64] = (SCALE * w_in[h * DH:(h + 1) * DH, :]).T
            wkvT[:, hl * 64:(hl + 1) * 64] = (SCALE * w_in[D + h * DH:D + (h + 1) * DH, :]).T
            wkvT[:, 256 + hl * 64:256 + (hl + 1) * 64] = w_in[2 * D + h * DH:2 * D + (h + 1) * DH, :].T
        woT = np.empty((256, D), dtype=np.float32)
        for hl, h in enumerate(heads):
            woT[hl * 64:(hl + 1) * 64, :] = w_out[:, h * DH:(h + 1) * DH].T
        betac = np.zeros((128, 20), dtype=np.float32)
        for hp in range(2):
            for who in range(2):
                h = heads[2 * hp + who]
                betac[:, hp * 10 + who * 5:hp * 10 + who * 5 + NF] = (
                    beta[h, 1:1 + NF] * pow4)[None, :]
        in_maps.append({
            'xT': xT[b],
            'wqT': wqT.astype(bf16),
            'wkvT': wkvT.astype(bf16),
            'woT': woT.astype(bf16),
            'betac': betac,
            'eye': eye,
        })

    res = bass_utils.run_bass_kernel_spmd(nc, in_maps, core_ids=list(range(NCORES)))
    global last_results
    last_results = res

    out = np.zeros((B, S, D), dtype=np.float32)
    for cid in range(NCORES):
        out[cid // 4] += res.results[cid]['outp'].astype(np.float32).T
    return out
